# revision 1
# baseline (speedup 1.0000x reference)
"""Trainium2 Bass kernel for nn_LDRFat (3-layer MLP forward).

reference: logits = relu((x @ W) @ fc_w.T + fc_b) @ logits_w.T + logits_b

Key algebraic optimization: (x @ W) @ fc_w.T == x @ (W @ fc_w.T).
Precomputing Wfc = W @ fc_w.T ([3072,512], 9.7 GFLOP) collapses the
dominant 309 GFLOP x@W matmul into a 51.5 GFLOP x@Wfc.

Sharding: data-parallel over batch for the main pass (2048 rows/core).
The Wfc precompute is sharded over W's rows (each core gets its own
Wshard input, 384 rows) and the 8 shards are combined with an AllGather
collective. Set KERNEL_V1=1 for the no-collective fallback (every core
redundantly computes all of Wfc from the full W input).

Matmuls run as float32r (FP22 multiply, fp32 accumulate) = full PE rate.
Transposes (PE transpose mode, plain fp32) are exact. Transposes are
batched into dense runs separate from matmul runs: PE transpose-mode
doesn't count as busy for the HAM clock gate, so interleaving T/MM kept
the PE at 1.2 GHz (measured 6x slowdown on phase B).
"""

import os
import numpy as np

import concourse.bass as bass
import concourse.mybir as mybir
import concourse.tile as tile
from concourse import bacc
from concourse.bass import MemorySpace, ts, ds
from concourse.bass_utils import run_bass_kernel_spmd
from concourse.masks import make_identity

B = 16384
N = 3072
FC = 512
CLS = 10
NCORES = 8
BS = B // NCORES   # 2048 rows per core
P = 128

KT = N // P        # 24 k-tiles
NT = N // P        # 24 n-tiles
FT = FC // P       # 4 f-tiles
MCHUNK = 512
NMC = BS // MCHUNK   # 4 m-chunks per core
MSUB = MCHUNK // P   # 4 sub-tiles per chunk
KSH = KT // NCORES   # 3 k-tiles per core in sharded precompute
WROWS = KSH * P      # 384 W-rows per core

F32 = mybir.dt.float32
F32R = mybir.dt.float32r

_CACHE = {}
LAST_RESULT = None


def _build_fcwT(nc, tc, ps_tp, fcw_d, identity, fcwT):
    """fc_wT[n, f] tiles via PE transposes (dense-batched)."""
    with tc.tile_pool(name="fcw_nat", bufs=2) as fcw_nat_pool:
        for ft in range(FT):
            fstrip = fcw_nat_pool.tile([P, N], F32, tag="fcwstrip")
            nc.sync.dma_start(fstrip, fcw_d[ts(ft, P), :])
            for nt in range(NT):
                pst = ps_tp.tile([P, P], F32, tag="tp")
                nc.tensor.transpose(pst, fstrip[:, ts(nt, P)], identity)
                nc.vector.tensor_copy(fcwT[:, nt, ts(ft, P)], pst)


def _wfc_shard_compute(nc, tc, ps_tp, ps_acc, w_src, fcwT, dst, nkt,
                       w_strip_pool, wTs_pool):
    """dst[:, lkt] = Wfc rows for k-tiles of w_src (nkt tiles)."""
    for lkt in range(nkt):
        wstrip = w_strip_pool.tile([P, N], F32, tag="wstrip")
        nc.sync.dma_start(wstrip, w_src[ts(lkt, P), :])
        wTs = wTs_pool.tile([P, NT, P], F32R, tag="wTs")
        for nt in range(NT):
            pst = ps_tp.tile([P, P], F32, tag="tp")
            nc.tensor.transpose(pst, wstrip[:, ts(nt, P)], identity_g[0])
            nc.vector.tensor_copy(wTs[:, nt], pst)
        acc = ps_acc.tile([P, FC], F32, tag="acc")
        for nt in range(NT):
            nc.tensor.matmul(
                acc, wTs[:, nt], fcwT[:, nt],
                start=(nt == 0), stop=(nt == NT - 1),
            )
        nc.vector.tensor_copy(dst[:, lkt], acc)


identity_g = [None]


def build_kernel(phase=None):
    phase = phase or os.environ.get("KERNEL_PHASE", "both")
    v1 = bool(int(os.environ.get("KERNEL_V1", "1")))
    repeat = int(os.environ.get("KERNEL_REPEAT", "1"))

    nc = bacc.Bacc(
        "TRN2",
        target_bir_lowering=False,
        debug=False,
        enable_asserts=False,
        num_devices=NCORES,
    )
    x_d = nc.dram_tensor("x", [BS, N], F32, kind="ExternalInput").ap()
    if v1:
        w_d = nc.dram_tensor("W", [N, N], F32, kind="ExternalInput").ap()
    else:
        wsh_d = nc.dram_tensor("Wshard", [WROWS, N], F32, kind="ExternalInput").ap()
    fcw_d = nc.dram_tensor("fc_w", [FC, N], F32, kind="ExternalInput").ap()
    fcb_d = nc.dram_tensor("fc_b", [FC], F32, kind="ExternalInput").ap()
    lgw_d = nc.dram_tensor("logits_w", [CLS, FC], F32, kind="ExternalInput").ap()
    lgb_d = nc.dram_tensor("logits_b", [CLS], F32, kind="ExternalInput").ap()
    out_d = nc.dram_tensor("out", [BS, CLS], F32, kind="ExternalOutput").ap()

    with tile.TileContext(nc) as tc:
        with (
            tc.tile_pool(name="consts", bufs=1) as consts,
            tc.tile_pool(name="wfc", bufs=1) as wfc_pool,
            tc.tile_pool(name="ps_acc", bufs=4, space=MemorySpace.PSUM) as ps_acc,
            tc.tile_pool(name="ps_tp", bufs=3, space=MemorySpace.PSUM) as ps_tp,
            tc.tile_pool(name="ps_lg", bufs=1, space=MemorySpace.PSUM) as ps_lg,
        ):
            identity = consts.tile([P, P], F32)
            make_identity(nc, identity)
            identity_g[0] = identity

            fcb_sb = consts.tile([P, FT], F32)
            nc.sync.dma_start(fcb_sb, fcb_d.rearrange("(t p) -> p t", p=P))

            lgw_sb = consts.tile([CLS, FC], F32)
            nc.sync.dma_start(lgw_sb, lgw_d)
            lgb_stage = consts.tile([1, CLS], F32)
            nc.sync.dma_start(lgb_stage, lgb_d.rearrange("(a c) -> a c", a=1))
            lgb_sb = consts.tile([1, CLS], F32R)
            nc.vector.tensor_copy(lgb_sb, lgb_stage)
            ones_stage = consts.tile([1, P], F32)
            nc.gpsimd.memset(ones_stage, 1.0)
            ones_sb = consts.tile([1, P], F32R)
            nc.vector.tensor_copy(ones_sb, ones_stage)

            lgwT_sb = consts.tile([P, FT, CLS], F32R)
            for ft in range(FT):
                pst = ps_tp.tile([P, P], F32, tag="tp")
                nc.tensor.transpose(
                    pst[:, :CLS], lgw_sb[:, ts(ft, P)], identity[:CLS, :CLS]
                )
                nc.vector.tensor_copy(lgwT_sb[:, ft], pst[:, :CLS])

            # Wfc[k, f] = sum_n W[k, n] fc_w[f, n]; resident all of phase B
            wfc_sb = wfc_pool.tile([P, KT, FC], F32R)

            # ---------------- Phase A ----------------
            if phase in ("both", "a") and not v1:
                # sharded precompute + AllGather
                with (
                    tc.tile_pool(name="fcwT_p", bufs=1) as fcwT_pool,
                    tc.tile_pool(name="w_strip", bufs=2) as w_strip_pool,
                    tc.tile_pool(name="wTs_p", bufs=2) as wTs_pool,
                    tc.tile_pool(name="wfc_stage", bufs=1) as wfc_stage_pool,
                    tc.tile_pool(name="cc_dram", bufs=1, space=MemorySpace.DRAM) as ccd,
                ):
                    fcwT = fcwT_pool.tile([P, NT, FC], F32R)
                    _build_fcwT(nc, tc, ps_tp, fcw_d, identity, fcwT)

                    wfc_stage = wfc_stage_pool.tile([P, KSH, FC], F32R)
                    _wfc_shard_compute(nc, tc, ps_tp, ps_acc, wsh_d, fcwT,
                                       wfc_stage, KSH, w_strip_pool, wTs_pool)

                    gin = ccd.tile([P, KSH * FC], F32R)
                    nc.sync.dma_start(
                        gin, wfc_stage.rearrange("p a b -> p (a b)")
                    )
                    gout = ccd.tile([NCORES * P, KSH * FC], F32R)
                    nc.gpsimd.collective_compute(
                        "AllGather",
                        mybir.AluOpType.bypass,
                        replica_groups=[list(range(NCORES))],
                        ins=[gin.opt()],
                        outs=[gout.opt()],
                    )
                    # gout rows = (core c, partition p); free j = (lkt, f)
                    nc.sync.dma_start(
                        wfc_sb.rearrange("p (c l) f -> p c (l f)", c=NCORES),
                        gout.rearrange("(c p) j -> p c j", p=P),
                    )

            if phase in ("both", "a") and v1:
                with (
                    tc.tile_pool(name="fcwT_p", bufs=1) as fcwT_pool,
                    tc.tile_pool(name="w_strip", bufs=2) as w_strip_pool,
                    tc.tile_pool(name="wTs_p", bufs=2) as wTs_pool,
                ):
                    fcwT = fcwT_pool.tile([P, NT, FC], F32R)
                    _build_fcwT(nc, tc, ps_tp, fcw_d, identity, fcwT)
                    for _arep in range(int(os.environ.get("KERNEL_REPEAT_A", "1"))):
                        _wfc_shard_compute(nc, tc, ps_tp, ps_acc, w_d, fcwT,
                                           wfc_sb, KT, w_strip_pool, wTs_pool)

            if phase == "b":
                nc.gpsimd.memset(wfc_sb.bitcast(F32), 0.0)
            if phase == "a":
                with tc.tile_pool(name="dbg_dram", bufs=1, space=MemorySpace.DRAM) as dp:
                    wfc_dump = dp.tile([P, KT * FC], F32)
                    nc.sync.dma_start(
                        wfc_dump, wfc_sb.bitcast(F32).rearrange("p a b -> p (a b)")
                    )
                    dump = consts.tile([P, CLS], F32)
                    nc.vector.tensor_copy(dump, wfc_sb[:, 0, :CLS].bitcast(F32))
                    nc.sync.dma_start(out_d[:P, :], dump)

            # ---------------- Phase B ----------------
            if phase in ("both", "b"):
                with (
                    tc.tile_pool(name="x_nat", bufs=5) as x_nat_pool,
                    tc.tile_pool(name="xT", bufs=1) as xT_pool,
                    tc.tile_pool(name="yT", bufs=2) as yT_pool,
                    tc.tile_pool(name="out_sb", bufs=3) as out_pool,
                ):
                    for rep in range(repeat):
                        for mc in range(NMC):
                            xs = []
                            for msub in range(MSUB):
                                xn = x_nat_pool.tile([P, N], F32, tag="xnat")
                                nc.sync.dma_start(
                                    xn, x_d[ds(mc * MCHUNK + msub * P, P), :]
                                )
                                xs.append(xn)

                            # dense transpose run for the whole chunk
                            xTs = xT_pool.tile([P, KT, MCHUNK], F32R, tag="xTs")
                            for kt in range(KT):
                                for msub in range(MSUB):
                                    pst = ps_tp.tile([P, P], F32, tag="tp")
                                    nc.tensor.transpose(
                                        pst, xs[msub][:, ts(kt, P)], identity
                                    )
                                    nc.vector.tensor_copy(
                                        xTs[:, kt, ts(msub, P)], pst
                                    )

                            # dense matmul run
                            h2 = [
                                ps_acc.tile(
                                    [P, MCHUNK], F32, tag="acc",
                                    name=f"h2_{rep}_{mc}_{ft}",
                                )
                                for ft in range(FT)
                            ]
                            for kt in range(KT):
                                for ft in range(FT):
                                    nc.tensor.matmul(
                                        h2[ft],
                                        wfc_sb[:, kt, ts(ft, P)],
                                        xTs[:, kt],
                                        start=(kt == 0),
                                        stop=(kt == KT - 1),
                                    )

                            # relu(h2 + fc_b), per-partition bias on ACT
                            yT = yT_pool.tile([P, FT, MCHUNK], F32R, tag="yT")
                            for ft in range(FT):
                                nc.scalar.activation(
                                    yT[:, ft],
                                    h2[ft],
                                    mybir.ActivationFunctionType.Relu,
                                    bias=fcb_sb[:, ds(ft, 1)],
                                )

                            # logits + bias (K=1 ones x logits_b matmul)
                            for msub in range(MSUB):
                                plg = ps_lg.tile([P, CLS], F32, tag="lg")
                                for ft in range(FT):
                                    nc.tensor.matmul(
                                        plg,
                                        yT[:, ft, ts(msub, P)],
                                        lgwT_sb[:, ft],
                                        start=(ft == 0),
                                        stop=False,
                                    )
                                nc.tensor.matmul(
                                    plg, ones_sb, lgb_sb, start=False, stop=True
                                )
                                osb = out_pool.tile([P, CLS], F32, tag="osb")
                                nc.vector.tensor_copy(osb, plg)
                                nc.sync.dma_start(
                                    out_d[ds(mc * MCHUNK + msub * P, P), :], osb
                                )

    nc.compile()
    return nc


def kernel(**inputs) -> np.ndarray:
    global LAST_RESULT
    if "nc" not in _CACHE:
        _CACHE["nc"] = build_kernel()
    nc = _CACHE["nc"]
    v1 = bool(int(os.environ.get("KERNEL_V1", "1")))

    x = np.ascontiguousarray(inputs["x"], dtype=np.float32)
    W = np.ascontiguousarray(inputs["W"], dtype=np.float32)
    fc_w = np.ascontiguousarray(inputs["fc_w"], dtype=np.float32)
    fc_b = np.ascontiguousarray(inputs["fc_b"], dtype=np.float32)
    lgw = np.ascontiguousarray(inputs["logits_w"], dtype=np.float32)
    lgb = np.ascontiguousarray(inputs["logits_b"], dtype=np.float32)

    in_maps = []
    for i in range(NCORES):
        m = {
            "x": x[i * BS : (i + 1) * BS],
            "fc_w": fc_w,
            "fc_b": fc_b,
            "logits_w": lgw,
            "logits_b": lgb,
        }
        if v1:
            m["W"] = W
        else:
            m["Wshard"] = np.ascontiguousarray(W[i * WROWS : (i + 1) * WROWS])
        in_maps.append(m)

    res = run_bass_kernel_spmd(
        nc,
        in_maps,
        core_ids=list(range(NCORES)),
        trace=bool(int(os.environ.get("KERNEL_TRACE", "0"))),
    )
    LAST_RESULT = res
    out = np.concatenate([r_["out"] for r_ in res.results], axis=0)
    return out



# revision 4
# speedup vs baseline: 1.8417x; 1.8417x over previous
"""Trainium2 Bass kernel for nn_LDRFat (3-layer MLP forward).

reference: logits = relu((x @ W) @ fc_w.T + fc_b) @ logits_w.T + logits_b

Algebraic optimization: (x @ W) @ fc_w.T == x @ (W @ fc_w.T).
Precomputing Wfc = W @ fc_w.T ([3072,512]) collapses the dominant
309-GFLOP x@W matmul into a 51.5-GFLOP x@Wfc (phase B).

Device layout strategy (all matmul operands bf16, f32 accumulate):
 - Host feeds pre-transposed operands (xT, WT-shard, fc_wT, logits_wT)
   so the device does ZERO transposes: every tensor lands in SBUF with
   the contraction dim on partitions. This removes the 1000+ PE
   transposes + DVE cast-copies that dominated the previous version.
 - Phase A (sharded over W's k-rows, 3 k-tiles/core): wfc_shard =
   W[kshard,:] @ fc_w.T via 72 MMs; shards combined with a bf16
   AllGather. KERNEL_V1=1 falls back to redundant full precompute.
 - Phase B (data-parallel over batch, 2048 rows/core): h2T[f,m] =
   wfc.T-tiles (stationary) x xT (moving, free=512). Loop order
   ft-outer / kt / mc-inner reuses each stationary tile across 4
   matmuls, amortizing LDWEIGHTS 4x. ACT applies fused bias+relu
   (per-partition bias, f on partitions). Logits computed as
   outT[cls,m] with the small logits_wT as the stationary operand
   (4 LDWEIGHTS per m-chunk instead of 64 overall) and bias added via
   a K=1 ones x logits_b matmul. Output returned as [10, 2048] per
   core and transposed on the host.
"""

import os
import numpy as np
import ml_dtypes

import concourse.bass as bass
import concourse.mybir as mybir
import concourse.tile as tile
from concourse import bacc
from concourse.bass import MemorySpace, ts, ds
from concourse.bass_utils import run_bass_kernel_spmd

B = 16384
N = 3072
FC = 512
CLS = 10
NCORES = 8
BS = B // NCORES     # 2048 batch rows per core
P = 128

KT = N // P          # 24 k-tiles
FT = FC // P         # 4 f-tiles
MC = 4               # m-chunks per core
MCH = BS // MC       # 512
KSH = KT // NCORES   # 3 k-tiles per core in sharded precompute
WK = KSH * P         # 384 W-rows per core

F32 = mybir.dt.float32
BF16 = mybir.dt.bfloat16
BF = ml_dtypes.bfloat16

_CACHE = {}
LAST_RESULT = None


def build_kernel():
    v1 = bool(int(os.environ.get("KERNEL_V1", "0")))

    nc = bacc.Bacc(
        "TRN2",
        target_bir_lowering=False,
        debug=False,
        enable_asserts=False,
        num_devices=NCORES,
    )
    xT_d = nc.dram_tensor("xT", [N, BS], BF16, kind="ExternalInput").ap()
    if v1:
        wts_d = nc.dram_tensor("WT", [N, N], BF16, kind="ExternalInput").ap()
    else:
        wts_d = nc.dram_tensor("WTs", [N, WK], BF16, kind="ExternalInput").ap()
    fcwT_d = nc.dram_tensor("fcwT", [N, FC], BF16, kind="ExternalInput").ap()
    fcb_d = nc.dram_tensor("fc_b", [FC], F32, kind="ExternalInput").ap()
    lgwT_d = nc.dram_tensor("lgwT", [FC, CLS], BF16, kind="ExternalInput").ap()
    lgb_d = nc.dram_tensor("lgb", [CLS], BF16, kind="ExternalInput").ap()
    out_d = nc.dram_tensor("out", [CLS, BS], F32, kind="ExternalOutput").ap()

    with tile.TileContext(nc) as tc:
        with (
            tc.tile_pool(name="consts", bufs=1) as consts,
            tc.tile_pool(name="wfc", bufs=1) as wfc_pool,
            tc.tile_pool(name="xt", bufs=1) as xt_pool,
            tc.tile_pool(name="yt", bufs=1) as yt_pool,
            tc.tile_pool(name="osb", bufs=1) as out_pool,
        ):
            # ---- constants / small inputs (issued first on sync queue) ----
            fcb_sb = consts.tile([P, FT], F32)
            nc.sync.dma_start(fcb_sb, fcb_d.rearrange("(t p) -> p t", p=P))
            lgwT_sb = consts.tile([P, FT, CLS], BF16)
            nc.sync.dma_start(lgwT_sb, lgwT_d.rearrange("(t p) c -> p t c", p=P))
            lgb_sb = consts.tile([1, CLS], BF16)
            nc.sync.dma_start(lgb_sb, lgb_d.rearrange("(a c) -> a c", a=1))
            ones_stage = consts.tile([1, MCH], F32)
            nc.gpsimd.memset(ones_stage, 1.0)
            ones_sb = consts.tile([1, MCH], BF16)
            nc.vector.tensor_copy(ones_sb, ones_stage)

            # ---- bulk inputs: phase-A operands first, then xT ----
            fcwT_sb = consts.tile([P, KT, FC], BF16)
            nc.sync.dma_start(fcwT_sb, fcwT_d.rearrange("(t p) f -> p t f", p=P))
            if not v1:
                wts_sb = consts.tile([P, KT, WK], BF16)
                nc.sync.dma_start(wts_sb, wts_d.rearrange("(t p) k -> p t k", p=P))
            xt_sb = xt_pool.tile([P, KT, BS], BF16)
            nc.sync.dma_start(xt_sb, xT_d.rearrange("(t p) m -> p t m", p=P))

            # wfc[k, f] resident for all of phase B
            wfc_sb = wfc_pool.tile([P, KT, FC], BF16)

            # ---------------- Phase A: wfc = W @ fc_w.T ----------------
            if v1:
                with (
                    tc.tile_pool(name="wtk", bufs=3) as wtk_pool,
                    tc.tile_pool(name="ps_a", bufs=2, space=MemorySpace.PSUM) as ps_a,
                ):
                    for kt in range(KT):
                        wtk = wtk_pool.tile([P, KT, P], BF16, tag="wtk")
                        nc.sync.dma_start(
                            wtk,
                            wts_d[:, ts(kt, P)].rearrange("(t p) k -> p t k", p=P),
                        )
                        acc = ps_a.tile([P, FC], F32, tag="acc")
                        for nt in range(KT):
                            nc.tensor.matmul(
                                acc, wtk[:, nt], fcwT_sb[:, nt],
                                start=(nt == 0), stop=(nt == KT - 1),
                            )
                        nc.vector.tensor_copy(wfc_sb[:, kt], acc)
            else:
                with (
                    tc.tile_pool(name="wstg", bufs=1) as stage_pool,
                    tc.tile_pool(name="ps_a", bufs=2, space=MemorySpace.PSUM) as ps_a,
                    tc.tile_pool(name="ccd", bufs=1, space=MemorySpace.DRAM) as ccd,
                ):
                    wfc_stage = stage_pool.tile([P, KSH, FC], BF16)
                    for lkt in range(KSH):
                        acc = ps_a.tile([P, FC], F32, tag="acc")
                        for nt in range(KT):
                            nc.tensor.matmul(
                                acc, wts_sb[:, nt, ts(lkt, P)], fcwT_sb[:, nt],
                                start=(nt == 0), stop=(nt == KT - 1),
                            )
                        nc.vector.tensor_copy(wfc_stage[:, lkt], acc)

                    gin = ccd.tile([P, KSH * FC], BF16)
                    nc.sync.dma_start(gin, wfc_stage.rearrange("p a b -> p (a b)"))
                    gout = ccd.tile(
                        [NCORES * P, KSH * FC], BF16, addr_space="Shared"
                    )
                    nc.gpsimd.collective_compute(
                        "AllGather",
                        mybir.AluOpType.bypass,
                        replica_groups=[list(range(NCORES))],
                        ins=[gin.opt()],
                        outs=[gout.opt()],
                    )
                    # gout rows = (core c, partition p); core c's shard is
                    # global k-tiles 3c..3c+2
                    nc.sync.dma_start(
                        wfc_sb.rearrange("p (c l) f -> p c (l f)", c=NCORES),
                        gout.rearrange("(c p) j -> p c j", p=P),
                    )

            # ---------------- Phase B: h2T = relu(wfc.T @ xT + b) ----------------
            with (
                tc.tile_pool(name="ps_b", bufs=6, space=MemorySpace.PSUM) as ps_b,
                tc.tile_pool(name="ps_lg", bufs=2, space=MemorySpace.PSUM) as ps_lg,
            ):
                out_sb = out_pool.tile([CLS, BS], F32)
                yts = []
                for ft in range(FT):
                    # 4 open accumulation groups; stationary wfc tile reused
                    # across the 4 m-chunks
                    ps = [
                        ps_b.tile([P, MCH], F32, tag="h2", name=f"h2_{ft}_{mc}")
                        for mc in range(MC)
                    ]
                    for kt in range(KT):
                        for mc in range(MC):
                            nc.tensor.matmul(
                                ps[mc],
                                wfc_sb[:, kt, ts(ft, P)],
                                xt_sb[:, kt, ts(mc, MCH)],
                                start=(kt == 0),
                                stop=(kt == KT - 1),
                            )
                    yt = yt_pool.tile([P, MC, MCH], BF16, tag=f"yt{ft}")
                    for mc in range(MC):
                        nc.scalar.activation(
                            yt[:, mc],
                            ps[mc],
                            mybir.ActivationFunctionType.Relu,
                            bias=fcb_sb[:, ds(ft, 1)],
                        )
                    yts.append(yt)

                # logits: outT[cls, m] per m-chunk; stationary = lgwT tiles
                for mc in range(MC):
                    plg = ps_lg.tile([CLS, MCH], F32, tag="lg")
                    for ft in range(FT):
                        nc.tensor.matmul(
                            plg,
                            lgwT_sb[:, ft],
                            yts[ft][:, mc],
                            start=(ft == 0),
                            stop=False,
                        )
                    nc.tensor.matmul(plg, lgb_sb, ones_sb, start=False, stop=True)
                    nc.vector.tensor_copy(out_sb[:, ts(mc, MCH)], plg)

                nc.sync.dma_start(out_d, out_sb)

    nc.compile()
    return nc


def prep_inputs(inputs):
    """Host-side layout marshaling: slice per core, pre-transpose, bf16."""
    v1 = bool(int(os.environ.get("KERNEL_V1", "0")))
    x = np.asarray(inputs["x"], dtype=np.float32)
    W = np.asarray(inputs["W"], dtype=np.float32)
    fc_w = np.asarray(inputs["fc_w"], dtype=np.float32)
    fc_b = np.ascontiguousarray(inputs["fc_b"], dtype=np.float32)
    lgw = np.asarray(inputs["logits_w"], dtype=np.float32)
    lgb = np.asarray(inputs["logits_b"], dtype=np.float32)

    xT = np.ascontiguousarray(x.astype(BF).T)        # [N, B]
    WT = np.ascontiguousarray(W.astype(BF).T)        # [N, N] rows=n, cols=k
    fcwT = np.ascontiguousarray(fc_w.astype(BF).T)   # [N, FC]
    lgwT = np.ascontiguousarray(lgw.astype(BF).T)    # [FC, CLS]
    lgb_bf = lgb.astype(BF)

    in_maps = []
    for i in range(NCORES):
        m = {
            "xT": np.ascontiguousarray(xT[:, i * BS : (i + 1) * BS]),
            "fcwT": fcwT,
            "fc_b": fc_b,
            "lgwT": lgwT,
            "lgb": lgb_bf,
        }
        if v1:
            m["WT"] = WT
        else:
            m["WTs"] = np.ascontiguousarray(WT[:, i * WK : (i + 1) * WK])
        in_maps.append(m)
    return in_maps


def kernel(**inputs) -> np.ndarray:
    global LAST_RESULT
    if "nc" not in _CACHE:
        _CACHE["nc"] = build_kernel()
    nc = _CACHE["nc"]

    in_maps = prep_inputs(inputs)
    res = run_bass_kernel_spmd(
        nc,
        in_maps,
        core_ids=list(range(NCORES)),
        trace=bool(int(os.environ.get("KERNEL_TRACE", "0"))),
    )
    LAST_RESULT = res
    # per-core out is [CLS, BS]; transpose back to [BS, CLS]
    out = np.concatenate(
        [np.ascontiguousarray(r_["out"].T) for r_ in res.results], axis=0
    )
    return out


# revision 5
# speedup vs baseline: 2.0556x; 1.1161x over previous
"""Trainium2 Bass kernel for nn_LDRFat (3-layer MLP forward).

reference: logits = relu((x @ W) @ fc_w.T + fc_b) @ logits_w.T + logits_b

Algebraic optimization: (x @ W) @ fc_w.T == x @ (W @ fc_w.T).
Precomputing Wfc = W @ fc_w.T ([3072,512]) collapses the dominant
309-GFLOP x@W matmul into a 51.5-GFLOP x@Wfc (phase B).

Device strategy (all matmul operands bf16, f32 accumulate):
 - Host feeds pre-transposed, pre-permuted operands laid out exactly as
   the SBUF tiles ([partition, free] contiguous), so every DMA moves
   large contiguous per-partition chunks at full HBM bandwidth and the
   device does ZERO transposes.
 - Phase A (sharded over W's k-rows, 3 k-tiles/core): wfc_shard =
   W[kshard,:] @ fc_w.T via 72 MMs, nt-outer with 3 open PSUM groups so
   compute starts after the first half-chunk of its inputs lands.
   Shards combined with a bf16 AllGather (staging DMAs on the scalar
   HWDGE queue so they never sit behind the big xT DMA on sync).
 - Phase B (data-parallel over batch, 2048 rows/core): h2T[f,m] =
   wfc-tiles (stationary) x xT (moving, free=512), ft-outer / kt / mc
   so each stationary tile serves 4 matmuls. ACT applies fused
   bias+relu. Logits computed as outT[cls,m] with logits_wT stationary
   and bias added via a K=1 ones x logits_b matmul; host transposes the
   [10, 2048] per-core result back.
"""

import os
import numpy as np
import ml_dtypes

import concourse.bass as bass
import concourse.mybir as mybir
import concourse.tile as tile
from concourse import bacc
from concourse.bass import MemorySpace, ts, ds
from concourse.bass_utils import run_bass_kernel_spmd

B = 16384
N = 3072
FC = 512
CLS = 10
NCORES = 8
BS = B // NCORES     # 2048 batch rows per core
P = 128

KT = N // P          # 24 k/n tiles
FT = FC // P         # 4 f-tiles
MC = 4               # m-chunks per core
MCH = BS // MC       # 512
KSH = KT // NCORES   # 3 k-tiles per core in sharded precompute
WK = KSH * P         # 384 W-rows per core
NCHUNK = 2           # phase-A input DMA chunks
NTC = KT // NCHUNK   # nt-tiles per chunk

F32 = mybir.dt.float32
BF16 = mybir.dt.bfloat16
BF = ml_dtypes.bfloat16

_CACHE = {}
LAST_RESULT = None


def build_kernel():
    nc = bacc.Bacc(
        "TRN2",
        target_bir_lowering=False,
        debug=False,
        enable_asserts=False,
        num_devices=NCORES,
    )
    # pre-permuted [partition, free] layouts (see prep_inputs)
    xT_d = nc.dram_tensor("xTr", [P, KT * BS], BF16, kind="ExternalInput").ap()
    wts_d = nc.dram_tensor("WTsr", [P, KT * WK], BF16, kind="ExternalInput").ap()
    fcwT_d = nc.dram_tensor("fcwTr", [P, KT * FC], BF16, kind="ExternalInput").ap()
    fcb_d = nc.dram_tensor("fc_b", [FC], F32, kind="ExternalInput").ap()
    lgwT_d = nc.dram_tensor("lgwT", [FC, CLS], BF16, kind="ExternalInput").ap()
    lgb_d = nc.dram_tensor("lgb", [CLS], BF16, kind="ExternalInput").ap()
    out_d = nc.dram_tensor("out", [CLS, BS], F32, kind="ExternalOutput").ap()

    with tile.TileContext(nc) as tc:
        with (
            tc.tile_pool(name="consts", bufs=1) as consts,
            tc.tile_pool(name="wfc", bufs=1) as wfc_pool,
            tc.tile_pool(name="xt", bufs=1) as xt_pool,
            tc.tile_pool(name="yt", bufs=1) as yt_pool,
            tc.tile_pool(name="osb", bufs=1) as out_pool,
        ):
            # ---- constants / small inputs (issued first on sync queue) ----
            fcb_sb = consts.tile([P, FT], F32)
            nc.sync.dma_start(fcb_sb, fcb_d.rearrange("(t p) -> p t", p=P))
            lgwT_sb = consts.tile([P, FT, CLS], BF16)
            nc.sync.dma_start(lgwT_sb, lgwT_d.rearrange("(t p) c -> p t c", p=P))
            lgb_sb = consts.tile([1, CLS], BF16)
            nc.sync.dma_start(lgb_sb, lgb_d.rearrange("(a c) -> a c", a=1))
            ones_stage = consts.tile([1, MCH], F32)
            nc.gpsimd.memset(ones_stage, 1.0)
            ones_sb = consts.tile([1, MCH], BF16)
            nc.vector.tensor_copy(ones_sb, ones_stage)

            # ---- bulk inputs: phase-A operands chunked first, then xT ----
            wts_sb = consts.tile([P, KT, WK], BF16)
            fcwT_sb = consts.tile([P, KT, FC], BF16)
            for c in range(NCHUNK):
                nc.sync.dma_start(
                    wts_sb[:, ts(c, NTC)].rearrange("p t k -> p (t k)"),
                    wts_d[:, ds(c * NTC * WK, NTC * WK)],
                )
                nc.sync.dma_start(
                    fcwT_sb[:, ts(c, NTC)].rearrange("p t f -> p (t f)"),
                    fcwT_d[:, ds(c * NTC * FC, NTC * FC)],
                )
            xt_sb = xt_pool.tile([P, KT, BS], BF16)
            nc.sync.dma_start(
                xt_sb.rearrange("p t m -> p (t m)"), xT_d
            )

            # wfc[k, f] resident for all of phase B
            wfc_sb = wfc_pool.tile([P, KT, FC], BF16)

            # ---------------- Phase A: wfc = W @ fc_w.T ----------------
            with (
                tc.tile_pool(name="wstg", bufs=1) as stage_pool,
                tc.tile_pool(name="ps_a", bufs=3, space=MemorySpace.PSUM) as ps_a,
                tc.tile_pool(name="ccd", bufs=1, space=MemorySpace.DRAM) as ccd,
            ):
                wfc_stage = stage_pool.tile([P, KSH, FC], BF16)
                accs = [
                    ps_a.tile([P, FC], F32, tag="acc", name=f"acc{lkt}")
                    for lkt in range(KSH)
                ]
                for nt in range(KT):
                    for lkt in range(KSH):
                        nc.tensor.matmul(
                            accs[lkt], wts_sb[:, nt, ts(lkt, P)], fcwT_sb[:, nt],
                            start=(nt == 0), stop=(nt == KT - 1),
                        )
                for lkt in range(KSH):
                    nc.vector.tensor_copy(wfc_stage[:, lkt], accs[lkt])

                # collective staging on the scalar HWDGE queue: never queued
                # behind the 12.6 MB xT DMA on sync
                gin = ccd.tile([P, KSH * FC], BF16)
                nc.scalar.dma_start(gin, wfc_stage.rearrange("p a b -> p (a b)"))
                gout = ccd.tile([NCORES * P, KSH * FC], BF16, addr_space="Shared")
                nc.gpsimd.collective_compute(
                    "AllGather",
                    mybir.AluOpType.bypass,
                    replica_groups=[list(range(NCORES))],
                    ins=[gin.opt()],
                    outs=[gout.opt()],
                )
                # gout rows = (core c, partition p); core c's shard is
                # global k-tiles 3c..3c+2
                nc.scalar.dma_start(
                    wfc_sb.rearrange("p (c l) f -> p c (l f)", c=NCORES),
                    gout.rearrange("(c p) j -> p c j", p=P),
                )

            # ------------ Phase B: h2T = relu(wfc.T @ xT + b) ------------
            with (
                tc.tile_pool(name="ps_b", bufs=6, space=MemorySpace.PSUM) as ps_b,
                tc.tile_pool(name="ps_lg", bufs=2, space=MemorySpace.PSUM) as ps_lg,
            ):
                out_sb = out_pool.tile([CLS, BS], F32)
                yts = []
                for ft in range(FT):
                    # 4 open accumulation groups; stationary wfc tile serves
                    # the 4 m-chunks
                    ps = [
                        ps_b.tile([P, MCH], F32, tag="h2", name=f"h2_{ft}_{mc}")
                        for mc in range(MC)
                    ]
                    for kt in range(KT):
                        for mc in range(MC):
                            nc.tensor.matmul(
                                ps[mc],
                                wfc_sb[:, kt, ts(ft, P)],
                                xt_sb[:, kt, ts(mc, MCH)],
                                start=(kt == 0),
                                stop=(kt == KT - 1),
                            )
                    yt = yt_pool.tile([P, MC, MCH], BF16, tag=f"yt{ft}")
                    for mc in range(MC):
                        nc.scalar.activation(
                            yt[:, mc],
                            ps[mc],
                            mybir.ActivationFunctionType.Relu,
                            bias=fcb_sb[:, ds(ft, 1)],
                        )
                    yts.append(yt)

                # logits: outT[cls, m] per m-chunk; stationary = lgwT tiles
                for mc in range(MC):
                    plg = ps_lg.tile([CLS, MCH], F32, tag="lg")
                    for ft in range(FT):
                        nc.tensor.matmul(
                            plg,
                            lgwT_sb[:, ft],
                            yts[ft][:, mc],
                            start=(ft == 0),
                            stop=False,
                        )
                    nc.tensor.matmul(plg, lgb_sb, ones_sb, start=False, stop=True)
                    nc.vector.tensor_copy(out_sb[:, ts(mc, MCH)], plg)

                nc.sync.dma_start(out_d, out_sb)

    nc.compile()
    return nc


def _permute(a2d, rows_per_tile=P):
    """[T*P, F] -> [P, T*F] so partition p's data is contiguous in DRAM."""
    t = a2d.shape[0] // rows_per_tile
    return np.ascontiguousarray(
        a2d.reshape(t, rows_per_tile, a2d.shape[1])
        .transpose(1, 0, 2)
        .reshape(rows_per_tile, t * a2d.shape[1])
    )


def prep_inputs(inputs):
    """Host-side layout marshaling: slice per core, pre-transpose, bf16."""
    x = np.asarray(inputs["x"], dtype=np.float32)
    W = np.asarray(inputs["W"], dtype=np.float32)
    fc_w = np.asarray(inputs["fc_w"], dtype=np.float32)
    fc_b = np.ascontiguousarray(inputs["fc_b"], dtype=np.float32)
    lgw = np.asarray(inputs["logits_w"], dtype=np.float32)
    lgb = np.asarray(inputs["logits_b"], dtype=np.float32)

    xT = x.astype(BF).T                              # [N, B] view
    WT = W.astype(BF).T                              # [N, N] rows=n, cols=k
    fcwTr = _permute(np.ascontiguousarray(fc_w.astype(BF).T))  # [P, KT*FC]
    lgwT = np.ascontiguousarray(lgw.astype(BF).T)    # [FC, CLS]
    lgb_bf = lgb.astype(BF)

    in_maps = []
    for i in range(NCORES):
        m = {
            "xTr": _permute(np.ascontiguousarray(xT[:, i * BS : (i + 1) * BS])),
            "WTsr": _permute(np.ascontiguousarray(WT[:, i * WK : (i + 1) * WK])),
            "fcwTr": fcwTr,
            "fc_b": fc_b,
            "lgwT": lgwT,
            "lgb": lgb_bf,
        }
        in_maps.append(m)
    return in_maps


def kernel(**inputs) -> np.ndarray:
    global LAST_RESULT
    if "nc" not in _CACHE:
        _CACHE["nc"] = build_kernel()
    nc = _CACHE["nc"]

    in_maps = prep_inputs(inputs)
    res = run_bass_kernel_spmd(
        nc,
        in_maps,
        core_ids=list(range(NCORES)),
        trace=bool(int(os.environ.get("KERNEL_TRACE", "0"))),
    )
    LAST_RESULT = res
    # per-core out is [CLS, BS]; transpose back to [BS, CLS]
    out = np.concatenate(
        [np.ascontiguousarray(r_["out"].T) for r_ in res.results], axis=0
    )
    return out
